# revision 8
# baseline (speedup 1.0000x reference)
"""Trainium2 Bass kernel for a 2-layer GCN (gnn_message_passing).

Strategy (8 NeuronCores, SPMD):
  - Destination nodes are balance-permuted and sharded across 8 cores
    (nb blocks of 128 dests per core). Weights replicated.
  - Layer tables (dinv-prescaled transformed features) are produced
    shard-wise and AllGathered so every core can gather any source row.
  - Aggregation uses the custom GPSIMD dma_gather (int16 indices, so the
    node table is split into 4 source chunks of <=32768 rows). Per
    128-dest block, the scatter-add is a PE matmul: a weighted one-hot
    [128 edges x 128 dests] built on DVE (is_equal+mult tensor_scalar
    against an iota row) multiplies the gathered source rows, accumulating
    in PSUM across the block's edge groups and chunks.
  - deg is computed on device by summing a host-padded per-dest edge-weight
    layout; dinv = 1/sqrt(deg) stays core-local.

Math (identical to the reference up to float assoc):
  x'' = dinv * (X @ W1)            (dinv folded into the gather table)
  h   = relu(dinv_d * sum_e ew_e * x''[src_e] + b1)
  h'' = dinv * h
  out = dinv_d * (sum_e ew_e * h''[src_e]) @ W2 + b2
"""

import sys
import types

if "/opt/trn_rl_repo" not in sys.path:
    sys.path.insert(0, "/opt/trn_rl_repo")

import numpy as np


def _install_ntff_shim():
    """antenv.axon_hooks is missing in this container; provide it so
    trace=True (NTFF profiling) works under axon."""
    if "antenv.axon_hooks" in sys.modules:
        return
    try:
        import antenv  # noqa: F401
    except ImportError:
        return
    shim = types.ModuleType("antenv.axon_hooks")
    shim._hook = None

    def set_axon_ntff_profile_hook(h):
        shim._hook = h

    def get_axon_ntff_profile_hook():
        return shim._hook

    shim.set_axon_ntff_profile_hook = set_axon_ntff_profile_hook
    shim.get_axon_ntff_profile_hook = get_axon_ntff_profile_hook
    sys.modules["antenv.axon_hooks"] = shim
    import antenv

    antenv.axon_hooks = shim
    try:
        from trn_agent_boot.trn_boot import _ntff_profile_via_ctypes

        shim._hook = _ntff_profile_via_ctypes("/opt/axon/libaxon_pjrt.so")
    except Exception:
        pass


N_CORES = 8
P = 128   # partitions / dest-block size
CH = 4    # source chunks (dma_gather indices are int16: chunk < 32768 rows)


class Cfg:
    def __init__(self, n_real, f_in, f_out, blocks_per_core, sb_blocks=7):
        self.n_real = n_real
        self.f = f_in          # hidden width (=64); gather elem = f*4 B
        self.f2 = f_out        # output width (=16)
        self.nb = blocks_per_core
        self.shard = blocks_per_core * P
        self.npad = N_CORES * self.shard
        self.sbb = sb_blocks   # dest blocks per gather superblock
        assert self.nb % self.sbb == 0
        self.chunk_rows = self.npad // CH
        assert self.chunk_rows * CH == self.npad and self.chunk_rows <= 32768
        assert self.npad >= n_real


def _host_prep(cfg, in_feat, edge_index, edge_weight):
    """Index/layout preprocessing. Float math here is limited to copying
    edge weights into padded layouts; reductions/matmuls run on device."""
    n, f = in_feat.shape
    assert n == cfg.n_real and f == cfg.f
    npad, nb, shard = cfg.npad, cfg.nb, cfg.shard
    nblocks = N_CORES * nb
    crows = cfg.chunk_rows

    src = np.asarray(edge_index[0], dtype=np.int64)
    dst = np.asarray(edge_index[1], dtype=np.int64)
    ew = np.asarray(edge_weight, dtype=np.float32)
    loop = np.arange(n, dtype=np.int64)   # self loops, weight 1
    src = np.concatenate([src, loop])
    dst = np.concatenate([dst, loop])
    ew = np.concatenate([ew, np.ones(n, np.float32)])

    # ---- balance-permute dest nodes into blocks of 128 slots -----------
    cnt = np.bincount(dst, minlength=npad)
    order = np.argsort(-cnt, kind="stable")
    pos = np.arange(npad, dtype=np.int64)
    pas = pos // nblocks
    bin_idx = pos % nblocks
    odd = (pas % 2) == 1
    bin_idx[odd] = nblocks - 1 - bin_idx[odd]
    nid = np.empty(npad, dtype=np.int64)
    nid[order] = bin_idx * P + pas

    dst_n = nid[dst]
    src_n = nid[src]
    L = int(cnt.max())

    # ---- per (core, block, chunk) edge segments ------------------------
    core_e = dst_n // shard
    b_e = (dst_n % shard) // P          # block within core
    k_e = src_n // crows                # source chunk
    # counts and static padded sizes R[b, k] (max over cores, %128)
    cnt_bk = np.zeros((N_CORES, nb, CH), np.int64)
    np.add.at(cnt_bk, (core_e, b_e, k_e), 1)
    R = ((cnt_bk.max(axis=0) + P - 1) // P * P).astype(np.int64)  # [nb, CH]
    Gbk = R // P

    # column layout: calls ordered (sb, k); within a call, blocks of the
    # superblock in order, Gbk[b,k] groups each. One global column space.
    sbb = cfg.sbb
    n_sb = nb // sbb
    call_cols = np.zeros((n_sb, CH), np.int64)     # start col of call
    col_of_bk = np.zeros((nb, CH), np.int64)       # start col of segment
    ccol = 0
    for s in range(n_sb):
        for k in range(CH):
            call_cols[s, k] = ccol
            for b in range(s * sbb, (s + 1) * sbb):
                col_of_bk[b, k] = ccol
                ccol += Gbk[b, k]
    G_total = ccol
    n_call = R.reshape(n_sb, sbb, CH).sum(axis=1)  # [n_sb, CH] num_idxs

    # ---- edge -> slot assignment ---------------------------------------
    # order edges by (core, block, chunk); rank within the segment
    key = (core_e * nb + b_e) * CH + k_e
    eorder = np.argsort(key, kind="stable")
    key_s = key[eorder]
    seg_start = np.searchsorted(key_s, np.arange(N_CORES * nb * CH))
    rank = np.arange(len(key_s)) - seg_start[key_s]

    core_s = core_e[eorder]
    b_s = b_e[eorder]
    k_s = k_e[eorder]
    dst_s = dst_n[eorder]
    src_s = src_n[eorder]
    ew_s = ew[eorder]

    slot_col = col_of_bk[b_s, k_s] + rank // P      # global group column
    slot_p = rank % P

    colrel_t = np.full((N_CORES, P, G_total), -1.0, np.float32)
    ew_t = np.zeros((N_CORES, P, G_total), np.float32)
    colrel_t[core_s, slot_p, slot_col] = (dst_s % P).astype(np.float32)
    ew_t[core_s, slot_p, slot_col] = ew_s

    # ---- int16 index tensor, call-major --------------------------------
    # call (s, k): slots i -> idx col off16 + i//16, partition i%16
    # (replicated over all 8 groups of 16 partitions)
    off16 = np.zeros((n_sb, CH), np.int64)
    o = 0
    for s in range(n_sb):
        for k in range(CH):
            off16[s, k] = o
            o += n_call[s, k] // 16
    idx_cols = o
    idx16 = np.zeros((N_CORES, 16, idx_cols), np.int16)
    # slot index within the call for each edge:
    #   (col_of_bk[b,k] - call_cols[s,k]) * 128 + rank_in_segment
    s_of_b = b_s // sbb
    slot_call = (col_of_bk[b_s, k_s] - call_cols[s_of_b, k_s]) * P \
        + rank // P * P + rank % P
    icol = off16[s_of_b, k_s] + slot_call // 16
    ip = slot_call % 16
    idx16[core_s, ip, icol] = (src_s - k_s * crows).astype(np.int16)
    idx16 = np.broadcast_to(idx16[:, None, :, :],
                            (N_CORES, 8, 16, idx_cols)).reshape(
                                N_CORES, P, idx_cols)

    # ---- deg layout -----------------------------------------------------
    dorder = np.argsort(dst_n, kind="stable")
    dst_d = dst_n[dorder]
    ew_d = ew[dorder]
    dfirst = np.searchsorted(dst_d, np.arange(npad))
    rankd = np.arange(len(dst_d)) - dfirst[dst_d]
    assert rankd.max() < L
    ewdeg = np.zeros((N_CORES, P, nb * L), np.float32)
    ewdeg[dst_d // shard, dst_d % P, ((dst_d % shard) // P) * L + rankd] = ew_d
    zdeg = np.where(cnt == 0)[0]
    if len(zdeg) > 0:
        zn = nid[zdeg]
        ewdeg[zn // shard, zn % P, ((zn % shard) // P) * L] = 1.0

    # ---- permuted, padded, transposed features -------------------------
    xperm = np.zeros((npad, f), np.float32)
    xperm[nid[:n]] = np.asarray(in_feat, np.float32)
    xt_shards = [
        np.ascontiguousarray(xperm[c * shard:(c + 1) * shard].T)
        for c in range(N_CORES)
    ]

    return dict(G_total=G_total, L=L, nid=nid, xt_shards=xt_shards,
                colrel_t=colrel_t, ew_t=ew_t, ewdeg=ewdeg, idx16=idx16,
                idx_cols=idx_cols, Gbk=Gbk, n_call=n_call, off16=off16,
                call_cols=call_cols, col_of_bk=col_of_bk)


def _build_program(cfg, prep):
    from concourse import bacc, mybir, tile

    f, f2, nb, shard, npad = cfg.f, cfg.f2, cfg.nb, cfg.shard, cfg.npad
    sbb, crows = cfg.sbb, cfg.chunk_rows
    n_sb = nb // sbb
    G_total, L, idx_cols = prep["G_total"], prep["L"], prep["idx_cols"]
    Gbk, n_call, off16 = prep["Gbk"], prep["n_call"], prep["off16"]
    call_cols = prep["call_cols"]
    fp32 = mybir.dt.float32
    Alu = mybir.AluOpType
    Act = mybir.ActivationFunctionType

    nc = bacc.Bacc("TRN2", target_bir_lowering=False, debug=False,
                   num_devices=N_CORES)

    xt_in = nc.dram_tensor("xt_shard", [f, shard], fp32, kind="ExternalInput")
    w1_in = nc.dram_tensor("w1", [f, f], fp32, kind="ExternalInput")
    w2_in = nc.dram_tensor("w2", [f, f2], fp32, kind="ExternalInput")
    b1_in = nc.dram_tensor("b1r", [P, f], fp32, kind="ExternalInput")
    b2_in = nc.dram_tensor("b2r", [P, f2], fp32, kind="ExternalInput")
    idx_in = nc.dram_tensor("idx", [P, idx_cols], mybir.dt.int16,
                            kind="ExternalInput")
    colrel_in = nc.dram_tensor("colrel", [P, G_total], fp32,
                               kind="ExternalInput")
    ew_in = nc.dram_tensor("ew", [P, G_total], fp32, kind="ExternalInput")
    ewdeg_in = nc.dram_tensor("ewdeg", [P, nb * L], fp32,
                              kind="ExternalInput")
    out_t = nc.dram_tensor("out", [shard, f2], fp32, kind="ExternalOutput")

    xw1_shard = nc.dram_tensor("xw1_shard", [shard, f], fp32, kind="Internal")
    xw1_full = nc.dram_tensor("xw1_full", [npad, f], fp32, kind="Internal",
                              addr_space="Shared")
    h2_shard = nc.dram_tensor("h2_shard", [shard, f], fp32, kind="Internal")
    h2_full = nc.dram_tensor("h2_full", [npad, f], fp32, kind="Internal",
                             addr_space="Shared")

    rg = [list(range(N_CORES))]
    # dma_gather faults beyond ~1024-1536 descriptors per call (SWDGE
    # descriptor-ring capacity); split calls into sub-calls of <= SUBI idxs.
    SUBI = 1024
    max_sub = max(
        (int(n_call[s, k]) + SUBI - 1) // SUBI
        for s in range(n_sb) for k in range(CH))

    with tile.TileContext(nc) as tc:
        with tc.tile_pool(name="const", bufs=1) as cpool, \
             tc.tile_pool(name="psum", bufs=sbb, space="PSUM") as pp, \
             tc.tile_pool(name="psum2", bufs=1, space="PSUM") as pp2, \
             tc.tile_pool(name="gather", bufs=1) as gpool, \
             tc.tile_pool(name="mask", bufs=6) as mpool, \
             tc.tile_pool(name="work", bufs=4) as wpool:

            # ---- constants / persistent state --------------------------
            iota_i = cpool.tile([P, P], mybir.dt.int32, name="iota_i")
            nc.gpsimd.iota(iota_i[:], pattern=[[1, P]], base=0,
                           channel_multiplier=0)
            iota_f = cpool.tile([P, P], fp32, name="iota_f")
            nc.vector.tensor_copy(out=iota_f[:], in_=iota_i[:])
            w1_sb = cpool.tile([f, f], fp32, name="w1_sb")
            nc.sync.dma_start(out=w1_sb[:], in_=w1_in[:])
            w2_sb = cpool.tile([f, f2], fp32, name="w2_sb")
            nc.sync.dma_start(out=w2_sb[:], in_=w2_in[:])
            b1_sb = cpool.tile([P, f], fp32, name="b1_sb")
            nc.sync.dma_start(out=b1_sb[:], in_=b1_in[:])
            b2_sb = cpool.tile([P, f2], fp32, name="b2_sb")
            nc.sync.dma_start(out=b2_sb[:], in_=b2_in[:])
            idx_sb = cpool.tile([P, idx_cols], mybir.dt.int16, name="idx_sb")
            nc.sync.dma_start(out=idx_sb[:], in_=idx_in[:])
            colrel_sb = cpool.tile([P, G_total], fp32, name="colrel_sb")
            nc.sync.dma_start(out=colrel_sb[:], in_=colrel_in[:])
            ew_sb = cpool.tile([P, G_total], fp32, name="ew_sb")
            nc.sync.dma_start(out=ew_sb[:], in_=ew_in[:])
            dinv_sb = cpool.tile([P, nb], fp32, name="dinv_sb")
            hpp_stage = cpool.tile([P, nb * f], fp32, name="hpp_stage")
            out_stage = cpool.tile([P, nb * f2], fp32, name="out_stage")

            # ---- deg -> dinv (core-local) ------------------------------
            with tc.tile_pool(name="deg", bufs=1) as degp:
                ewdeg_sb = degp.tile([P, nb * L], fp32)
                nc.sync.dma_start(out=ewdeg_sb[:], in_=ewdeg_in[:])
                deg_sb = degp.tile([P, nb], fp32)
                for b in range(nb):
                    nc.vector.reduce_sum(
                        out=deg_sb[:, b:b + 1],
                        in_=ewdeg_sb[:, b * L:(b + 1) * L],
                        axis=mybir.AxisListType.X)
                sq_sb = degp.tile([P, nb], fp32)
                nc.scalar.activation(out=sq_sb[:], in_=deg_sb[:],
                                     func=Act.Sqrt)
                nc.vector.reciprocal(out=dinv_sb[:], in_=sq_sb[:])

            # ---- dense: x'' = dinv * (X @ W1) for my shard rows --------
            SCH = 14 if nb % 14 == 0 else (7 if nb % 7 == 0 else
                                           (2 if nb % 2 == 0 else 1))
            XTW = 2048 if shard >= 2048 else shard
            with tc.tile_pool(name="xt", bufs=1) as xtp, \
                 tc.tile_pool(name="dst", bufs=2) as dstp:
                n_xt = (shard + XTW - 1) // XTW
                xt_tiles = []
                for i in range(n_xt):
                    w = min(XTW, shard - i * XTW)
                    xt = xtp.tile([f, XTW], fp32, name=f"xt{i}", tag=f"xt{i}")
                    nc.sync.dma_start(out=xt[:, :w],
                                      in_=xt_in[:, i * XTW:i * XTW + w])
                    xt_tiles.append(xt)
                for t0 in range(0, nb, SCH):
                    stg = dstp.tile([P, SCH * f], fp32, tag="dstg",
                                    name="dstg")
                    for j in range(SCH):
                        t = t0 + j
                        ps = pp.tile([P, f], fp32, tag="agg", name="dps")
                        lo = t * P
                        xt = xt_tiles[lo // XTW]
                        loc = lo % XTW
                        nc.tensor.matmul(out=ps[:], lhsT=xt[:, loc:loc + P],
                                         rhs=w1_sb[:], start=True, stop=True)
                        nc.vector.tensor_scalar(
                            out=stg[:, j * f:(j + 1) * f], in0=ps[:],
                            scalar1=dinv_sb[:, t:t + 1], scalar2=None,
                            op0=Alu.mult)
                    dst_ap = xw1_shard[t0 * P:(t0 + SCH) * P, :].rearrange(
                        "(i p) f -> p i f", p=P)
                    nc.sync.dma_start(out=dst_ap, in_=stg[:])

            nc.gpsimd.collective_compute(
                "AllGather", Alu.bypass, replica_groups=rg,
                ins=[xw1_shard[:]], outs=[xw1_full[:]])

            # ---- aggregation layers ------------------------------------
            def agg_layer(table, layer, stage_sb):
                for s in range(n_sb):
                    gts = {}  # (k, sub) -> tile
                    for k in range(CH):
                        n_i = int(n_call[s, k])
                        o16 = int(off16[s, k])
                        for sub in range((n_i + SUBI - 1) // SUBI):
                            m = min(SUBI, n_i - sub * SUBI)
                            gt = gpool.tile(
                                [P, (SUBI // P) * f], fp32,
                                tag=f"gt{k}_{sub}",
                                name=f"gt{layer}_{s}_{k}_{sub}")
                            nc.gpsimd.dma_gather(
                                gt[:, :m // P * f].rearrange(
                                    "p (a q) -> p a q", q=f),
                                table[k * crows:(k + 1) * crows, :],
                                idx_sb[:, o16 + sub * (SUBI // 16):
                                       o16 + sub * (SUBI // 16) + m // 16],
                                m, m, f)
                            gts[(k, sub)] = gt
                    # per-block PSUM accumulation across chunks/groups
                    pss = {}
                    nmm = {b: int(Gbk[b].sum())
                           for b in range(s * sbb, (s + 1) * sbb)}
                    done = {b: 0 for b in nmm}
                    for k in range(CH):
                        loc_col = 0
                        for b in range(s * sbb, (s + 1) * sbb):
                            for g in range(int(Gbk[b, k])):
                                gi = int(prep["col_of_bk"][b, k]) + g
                                mask = mpool.tile([P, P], fp32, tag="mask",
                                                  name="mask")
                                nc.vector.tensor_scalar(
                                    out=mask[:], in0=iota_f[:],
                                    scalar1=colrel_sb[:, gi:gi + 1],
                                    scalar2=ew_sb[:, gi:gi + 1],
                                    op0=Alu.is_equal, op1=Alu.mult)
                                if b not in pss:
                                    if layer == 1:
                                        pss[b] = pp.tile([P, f], fp32,
                                                         tag="agg",
                                                         name="aps")
                                    else:
                                        pss[b] = pp.tile([f, P], fp32,
                                                         tag="agg",
                                                         name="apsT")
                                sub, lc = divmod(loc_col, SUBI // P)
                                msg = gts[(k, sub)][:, lc * f:(lc + 1) * f]
                                first = done[b] == 0
                                last = done[b] == nmm[b] - 1
                                if layer == 1:
                                    nc.tensor.matmul(out=pss[b][:],
                                                     lhsT=mask[:], rhs=msg,
                                                     start=first, stop=last)
                                else:
                                    nc.tensor.matmul(out=pss[b][:],
                                                     lhsT=msg, rhs=mask[:],
                                                     start=first, stop=last)
                                done[b] += 1
                                loc_col += 1
                    for b in range(s * sbb, (s + 1) * sbb):
                        ps = pss[b]
                        if layer == 1:
                            t1 = wpool.tile([P, f], fp32, tag="t1", name="t1")
                            nc.vector.tensor_scalar(
                                out=t1[:], in0=ps[:],
                                scalar1=dinv_sb[:, b:b + 1], scalar2=None,
                                op0=Alu.mult)
                            t2 = wpool.tile([P, f], fp32, tag="t2", name="t2")
                            nc.vector.tensor_tensor(
                                out=t2[:], in0=t1[:], in1=b1_sb[:],
                                op=Alu.add)
                            nc.scalar.activation(
                                out=stage_sb[:, b * f:(b + 1) * f],
                                in_=t2[:], func=Act.Relu,
                                scale=dinv_sb[:, b:b + 1])
                        else:
                            lh = wpool.tile([f, P], fp32, tag="lh", name="lh")
                            nc.vector.tensor_copy(out=lh[:], in_=ps[:])
                            ps2 = pp2.tile([P, f2], fp32, tag="head",
                                           name="ps2")
                            nc.tensor.matmul(out=ps2[:], lhsT=lh[:],
                                             rhs=w2_sb[:], start=True,
                                             stop=True)
                            t3 = wpool.tile([P, f2], fp32, tag="t3",
                                            name="t3")
                            nc.vector.tensor_scalar(
                                out=t3[:], in0=ps2[:],
                                scalar1=dinv_sb[:, b:b + 1], scalar2=None,
                                op0=Alu.mult)
                            nc.vector.tensor_tensor(
                                out=stage_sb[:, b * f2:(b + 1) * f2],
                                in0=t3[:], in1=b2_sb[:], op=Alu.add)

            agg_layer(xw1_full, 1, hpp_stage)
            for t0 in range(0, nb, SCH):
                dst_ap = h2_shard[t0 * P:(t0 + SCH) * P, :].rearrange(
                    "(i p) f -> p i f", p=P)
                nc.sync.dma_start(out=dst_ap,
                                  in_=hpp_stage[:, t0 * f:(t0 + SCH) * f])
            nc.gpsimd.collective_compute(
                "AllGather", Alu.bypass, replica_groups=rg,
                ins=[h2_shard[:]], outs=[h2_full[:]])

            agg_layer(h2_full, 2, out_stage)
            out_ap = out_t[:].rearrange("(b p) f -> p b f", p=P)
            nc.sync.dma_start(out=out_ap, in_=out_stage[:])

    nc.compile()
    return nc


def _make_in_maps(cfg, prep, W1, b1, W2, b2):
    b1r = np.broadcast_to(np.asarray(b1, np.float32), (P, cfg.f)).copy()
    b2r = np.broadcast_to(np.asarray(b2, np.float32), (P, cfg.f2)).copy()
    w1 = np.asarray(W1, np.float32)
    w2 = np.asarray(W2, np.float32)
    in_maps = []
    for c in range(N_CORES):
        in_maps.append({
            "xt_shard": prep["xt_shards"][c],
            "w1": w1, "w2": w2, "b1r": b1r, "b2r": b2r,
            "idx": np.ascontiguousarray(prep["idx16"][c]),
            "colrel": np.ascontiguousarray(prep["colrel_t"][c]),
            "ew": np.ascontiguousarray(prep["ew_t"][c]),
            "ewdeg": np.ascontiguousarray(prep["ewdeg"][c]),
        })
    return in_maps


def run(cfg, in_feat, edge_index, edge_weight, W1, b1, W2, b2,
        trace=False, use_sim=False):
    """Returns (output [n_real, f2], BassKernelResults|None)."""
    _install_ntff_shim()
    from concourse import bass_utils

    prep = _host_prep(cfg, in_feat, edge_index, edge_weight)
    nc = _build_program(cfg, prep)
    in_maps = _make_in_maps(cfg, prep, W1, b1, W2, b2)

    if use_sim:
        from concourse.bass_interp import MultiCoreSim
        sim = MultiCoreSim(nc, num_cores=N_CORES)
        for c, (cid, core) in enumerate(sim.cores.items()):
            for k, v in in_maps[c].items():
                core.tensor(k)[:] = v
        sim.simulate()
        shards = [sim.cores[c].tensor("out").copy() for c in sim.cores]
        res = None
    else:
        res = bass_utils.run_bass_kernel_spmd(
            nc, in_maps, core_ids=list(range(N_CORES)), trace=trace)
        shards = [res.results[c]["out"] for c in range(N_CORES)]

    out_perm = np.concatenate(shards, axis=0)  # [npad, f2]
    out = out_perm[prep["nid"][:cfg.n_real]]
    return out, res


def kernel(in_feat, edge_index, edge_weight, W1, b1, W2, b2):
    cfg = Cfg(n_real=100000, f_in=64, f_out=16, blocks_per_core=98)
    out, _ = run(cfg, in_feat, edge_index, edge_weight, W1, b1, W2, b2)
    return np.ascontiguousarray(out.astype(np.float32))
